# revision 4
# baseline (speedup 1.0000x reference)
"""Trainium2 Bass kernel for nn_ConstrainedRNN (B=128,T=1024,I=256,H=512,C=32).

Strategy (data-parallel over batch, 8 cores x 16 sequences):
  - Fold constraints/biases into one augmented input projection:
      pre[b,t,:] = [x[b,t] | constraints[b] | 1 | 0pad] @ W_cat.T
    computed as a large fp32r matmul (phase A) into DRAM scratch.
  - Sequential scan (phase B) in transposed layout hT[j, b]:
      hT_{t+1} = tanh(W_hhT-tiles (stationary, bf16) @ hT_t + xp_t)
    16 (ldweights+matmul) pairs per step with N=16 moving columns.
  - Masking / `last` extraction handled on host from `lengths` (positions
    t >= len never feed back into any used value, so the scan runs unmasked).
"""
import sys
import numpy as np

for _p in ("/opt/trn_rl_repo", "/root/.axon_site/_ro/trn_rl_repo"):
    if _p not in sys.path:
        sys.path.insert(0, _p)

import ml_dtypes
import concourse.bass as bass
import concourse.mybir as mybir
import concourse.tile as tile
from concourse import bacc
from concourse.bass_utils import run_bass_kernel_spmd

F32 = mybir.dt.float32
F32R = mybir.dt.float32r
BF16 = mybir.dt.bfloat16
FP16 = mybir.dt.float16
TANH = mybir.ActivationFunctionType.Tanh

B, T_FULL, I, H, C = 128, 1024, 256, 512, 32
NCORES = 8
BC = B // NCORES          # 16 sequences per core
NJ = H // 128             # 4 output-row chunks
KP = 3                    # projection contraction chunks (384 = 289 padded)
KCAT = KP * 128           # 384
WIN = 32                  # scan steps per window (= one 512-col projection block)

# dtype of the recurrent weight/moving operand:
#   "fp16" (fast, ~5e-4 weight rounding), "bf16" (fast, ~4e-3), "f32" (exact, slow)
SCAN_DT = "fp16"
_SCAN_MDT = {"fp16": FP16, "bf16": BF16, "f32": F32}


def build(t_steps=T_FULL):
    """Build the SPMD Bass module for one core (t_steps scan steps)."""
    cols = t_steps * BC
    nb = cols // 512          # number of 512-column blocks == number of windows
    assert t_steps % WIN == 0

    nc = bacc.Bacc("TRN2", target_bir_lowering=False)
    xcatT = nc.dram_tensor("xcatT", [KP, 128, cols], F32, kind="ExternalInput")
    wcat = nc.dram_tensor("wcat", [128, KP * NJ * 128], F32, kind="ExternalInput")
    whh_dt = _SCAN_MDT[SCAN_DT]
    whh = nc.dram_tensor("whh", [128, 16 * 128], whh_dt, kind="ExternalInput")
    out_scr = nc.dram_tensor("out_scr", [128, t_steps, NJ * BC], F32,
                             kind="ExternalOutput")

    with tile.TileContext(nc) as tc:
        with (
            tc.tile_pool(name="wpool", bufs=1) as wpool,
            tc.tile_pool(name="xtp", bufs=2 * KP) as xtp,
            tc.tile_pool(name="psA", bufs=4, space="PSUM") as psA,
            tc.tile_pool(name="dramp", bufs=nb, space="DRAM") as dramp,
            tc.tile_pool(name="xwp", bufs=2) as xwp,
            tc.tile_pool(name="orp", bufs=2) as orp,
            tc.tile_pool(name="psB", bufs=4, space="PSUM") as psB,
            tc.tile_pool(name="hp", bufs=3) as hp,
            tc.tile_pool(name="prep", bufs=8) as prep,
        ):
            # ---- persistent weights ----
            wc = wpool.tile([128, KP * NJ * 128], F32R, tag="wc")
            nc.gpsimd.dma_start(wc[:], wcat[:])          # f32 -> f32r cast DMA
            wh = wpool.tile([128, 16 * 128], whh_dt, tag="wh")
            nc.sync.dma_start(wh[:], whh[:])

            # ---- phase A: projection xp[jc, p, n] per 512-col block ----
            xps = []
            for ib in range(nb):
                xt = []
                for kc in range(KP):
                    xk = xtp.tile([128, 512], F32R, tag=f"xt{kc}")
                    nc.gpsimd.dma_start(
                        xk[:], xcatT[kc, :, ib * 512:(ib + 1) * 512])
                    xt.append(xk)
                scr = dramp.tile([NJ, 128, 512], F32, tag="xps")
                for jc in range(NJ):
                    pp = psA.tile([128, 512], F32, tag="ppA")
                    for kc in range(KP):
                        nc.tensor.matmul(
                            pp[:], wc[:, (kc * NJ + jc) * 128:(kc * NJ + jc + 1) * 128],
                            xt[kc][:], start=(kc == 0), stop=(kc == KP - 1))
                    stg = xtp.tile([128, 512], F32, tag=f"stgA{jc % 2}")
                    nc.vector.tensor_copy(stg[:], pp[:])
                    nc.sync.dma_start(scr[jc, :, :], stg[:])
                xps.append(scr)

            # ---- phase B: scan ----
            h_dt = _SCAN_MDT[SCAN_DT]
            hprev = hp.tile([128, NJ * BC], h_dt, tag="hmov")
            nc.vector.memset(hprev[:], 0.0)
            for w in range(nb):
                xw = xwp.tile([128, NJ * 512], F32, tag="xw")
                nc.sync.dma_start(xw[:], xps[w][:].rearrange("j p n -> p j n"))
                oring = orp.tile([128, WIN * NJ * BC], F32, tag="oring")
                for trel in range(WIN):
                    P = psB.tile([128, NJ * BC], F32, tag="ppB")
                    for jc in range(NJ):
                        for kc in range(NJ):
                            nc.tensor.matmul(
                                P[:, jc * BC:(jc + 1) * BC],
                                wh[:, (kc * NJ + jc) * 128:(kc * NJ + jc + 1) * 128],
                                hprev[:, kc * BC:(kc + 1) * BC],
                                start=(kc == 0), stop=(kc == NJ - 1))
                    hnew = hp.tile([128, NJ * BC], h_dt, tag="hmov")
                    for jc in range(NJ):
                        pre = prep.tile([128, BC], F32, tag="pre")
                        nc.vector.tensor_add(
                            pre[:], P[:, jc * BC:(jc + 1) * BC],
                            xw[:, jc * 512 + trel * BC: jc * 512 + (trel + 1) * BC])
                        hs = oring[:, trel * NJ * BC + jc * BC:
                                   trel * NJ * BC + (jc + 1) * BC]
                        nc.scalar.activation(hs, pre[:], TANH)
                        nc.vector.tensor_copy(hnew[:, jc * BC:(jc + 1) * BC], hs)
                    hprev = hnew
                nc.sync.dma_start(
                    out_scr[:, w * WIN:(w + 1) * WIN, :], oring[:])
    nc.compile()
    return nc


def host_prep(x, constraints, W_ih, b_ih, W_hh, b_hh, W_ch, b_ch, t_steps=T_FULL):
    """Build per-core input maps."""
    x = np.asarray(x, dtype=np.float32)
    constraints = np.asarray(constraints, dtype=np.float32)
    bsz = x.shape[0]
    ncores_used = bsz // BC

    # augmented input  [b, t, 384]
    xcat = np.zeros((bsz, t_steps, KCAT), dtype=np.float32)
    xcat[:, :, :I] = x[:, :t_steps]
    xcat[:, :, I:I + C] = constraints[:, None, :]
    xcat[:, :, I + C] = 1.0

    # augmented weight [H, 384], tiles -> [128, KP*NJ*128]
    bias = (np.asarray(b_ih) + np.asarray(b_hh) + np.asarray(b_ch)).astype(np.float32)
    W_cat = np.zeros((H, KCAT), dtype=np.float32)
    W_cat[:, :I] = W_ih
    W_cat[:, I:I + C] = W_ch
    W_cat[:, I + C] = bias
    # wcat[p, kc*NJ+jc, m] = W_cat[jc*128+m, kc*128+p]
    wcat = np.ascontiguousarray(
        W_cat.T.reshape(KP, 128, NJ, 128).transpose(1, 0, 2, 3)
    ).reshape(128, KP * NJ * 128)

    # whh[p, kc*NJ+jc, m] = W_hh[jc*128+m, kc*128+p]
    whh = np.ascontiguousarray(
        np.asarray(W_hh, dtype=np.float32).T.reshape(NJ, 128, NJ, 128)
        .transpose(1, 0, 2, 3)
    ).reshape(128, 16 * 128)
    if SCAN_DT == "bf16":
        whh = whh.astype(ml_dtypes.bfloat16)
    elif SCAN_DT == "fp16":
        whh = whh.astype(np.float16)

    in_maps = []
    for c in range(ncores_used):
        xc = xcat[c * BC:(c + 1) * BC]                      # [16, t, 384]
        # xcatT[kc, ki, t*BC+bl] = xc[bl, t, kc*128+ki]
        xcT = np.ascontiguousarray(
            xc.reshape(BC, t_steps, KP, 128).transpose(2, 3, 1, 0)
        ).reshape(KP, 128, t_steps * BC)
        in_maps.append({"xcatT": xcT, "wcat": wcat, "whh": whh})
    return in_maps


def host_post(results, lengths, t_steps=T_FULL):
    """results: list of per-core dicts with out_scr [128, t, NJ*BC]."""
    lengths = np.asarray(lengths)
    outs = []
    for r in results:
        o = r["out_scr"]                                    # [128, t, 4*16]
        o = o.reshape(128, t_steps, NJ, BC).transpose(3, 1, 2, 0)  # [bl,t,jc,p]
        outs.append(o.reshape(BC, t_steps, H))
    outputs = np.ascontiguousarray(np.concatenate(outs, axis=0), dtype=np.float32)
    bsz = outputs.shape[0]
    mask = (np.arange(t_steps)[None, :] < lengths[:bsz, None])
    outputs *= mask[:, :, None].astype(np.float32)
    last = outputs[np.arange(bsz), np.clip(lengths[:bsz], 1, t_steps) - 1]
    return outputs, np.ascontiguousarray(last, dtype=np.float32)


_NC_CACHE = {}


def _get_nc(t_steps):
    if t_steps not in _NC_CACHE:
        _NC_CACHE[t_steps] = build(t_steps)
    return _NC_CACHE[t_steps]


def run(x, constraints, lengths, W_ih, b_ih, W_hh, b_hh, W_ch, b_ch,
        t_steps=T_FULL, trace=False):
    nc = _get_nc(t_steps)
    in_maps = host_prep(x, constraints, W_ih, b_ih, W_hh, b_hh, W_ch, b_ch,
                        t_steps)
    res = run_bass_kernel_spmd(nc, in_maps, core_ids=list(range(len(in_maps))),
                               trace=trace)
    outputs, last = host_post(res.results, lengths, t_steps)
    return outputs, last, res


def kernel(x, constraints, lengths, W_ih, b_ih, W_hh, b_hh, W_ch, b_ch):
    outputs, last, _ = run(x, constraints, lengths, W_ih, b_ih, W_hh, b_hh,
                           W_ch, b_ch)
    return outputs, last


# revision 6
# speedup vs baseline: 1.0733x; 1.0733x over previous
"""Trainium2 Bass kernel for nn_ConstrainedRNN (B=128,T=1024,I=256,H=512,C=32).

Strategy (data-parallel over batch, 8 cores x 16 sequences):
  - Fold constraints/biases into one augmented input projection:
      pre[b,t,:] = [x[b,t] | constraints[b] | 1 | 0pad] @ W_cat.T
    computed as a large fp32r matmul (phase A) into DRAM scratch.
  - Sequential scan (phase B) in transposed layout hT[j, b]:
      hT_{t+1} = tanh(W_hhT-tiles (stationary, bf16) @ hT_t + xp_t)
    16 (ldweights+matmul) pairs per step with N=16 moving columns.
  - Masking / `last` extraction handled on host from `lengths` (positions
    t >= len never feed back into any used value, so the scan runs unmasked).
"""
import sys
import numpy as np

for _p in ("/opt/trn_rl_repo", "/root/.axon_site/_ro/trn_rl_repo"):
    if _p not in sys.path:
        sys.path.insert(0, _p)

import ml_dtypes
import concourse.bass as bass
import concourse.mybir as mybir
import concourse.tile as tile
from concourse import bacc
from concourse.bass_utils import run_bass_kernel_spmd

F32 = mybir.dt.float32
F32R = mybir.dt.float32r
BF16 = mybir.dt.bfloat16
FP16 = mybir.dt.float16
TANH = mybir.ActivationFunctionType.Tanh

B, T_FULL, I, H, C = 128, 1024, 256, 512, 32
NCORES = 8
BC = B // NCORES          # 16 sequences per core
NJ = H // 128             # 4 output-row chunks
KP = 3                    # projection contraction chunks (384 = 289 padded)
KCAT = KP * 128           # 384
WIN = 32                  # scan steps per window (= one 512-col projection block)

# dtype of the recurrent weight/moving operand:
#   "fp16" (fast, ~5e-4 weight rounding), "bf16" (fast, ~4e-3), "f32" (exact, slow)
SCAN_DT = "fp16"
_SCAN_MDT = {"fp16": FP16, "bf16": BF16, "f32": F32}
ELEM_CHAINS = 2           # elementwise chains per step (1, 2, or 4)


def build(t_steps=T_FULL):
    """Build the SPMD Bass module for one core (t_steps scan steps)."""
    cols = t_steps * BC
    nb = cols // 512          # number of 512-column blocks == number of windows
    assert t_steps % WIN == 0

    nc = bacc.Bacc("TRN2", target_bir_lowering=False)
    xcatT = nc.dram_tensor("xcatT", [KP, 128, cols], F32, kind="ExternalInput")
    wcat = nc.dram_tensor("wcat", [128, KP * NJ * 128], F32, kind="ExternalInput")
    whh_dt = _SCAN_MDT[SCAN_DT]
    whh = nc.dram_tensor("whh", [128, 16 * 128], whh_dt, kind="ExternalInput")
    out_scr = nc.dram_tensor("out_scr", [128, t_steps, NJ * BC],
                             _SCAN_MDT[SCAN_DT], kind="ExternalOutput")

    with tile.TileContext(nc) as tc:
        with (
            tc.tile_pool(name="wpool", bufs=1) as wpool,
            tc.tile_pool(name="xtp", bufs=2 * KP) as xtp,
            tc.tile_pool(name="psA", bufs=4, space="PSUM") as psA,
            tc.tile_pool(name="dramp", bufs=nb, space="DRAM") as dramp,
            tc.tile_pool(name="xwp", bufs=2) as xwp,
            tc.tile_pool(name="orp", bufs=2) as orp,
            tc.tile_pool(name="psB", bufs=4, space="PSUM") as psB,
            tc.tile_pool(name="hp", bufs=1) as hp,
        ):
            # ---- persistent weights ----
            wc = wpool.tile([128, KP * NJ * 128], F32R, tag="wc")
            nc.gpsimd.dma_start(wc[:], wcat[:])          # f32 -> f32r cast DMA
            wh = wpool.tile([128, 16 * 128], whh_dt, tag="wh")
            nc.sync.dma_start(wh[:], whh[:])

            # ---- phase A: projection xp[jc, p, n] per 512-col block ----
            xps = []
            for ib in range(nb):
                xt = []
                for kc in range(KP):
                    xk = xtp.tile([128, 512], F32R, tag=f"xt{kc}")
                    nc.gpsimd.dma_start(
                        xk[:], xcatT[kc, :, ib * 512:(ib + 1) * 512])
                    xt.append(xk)
                scr = dramp.tile([NJ, 128, 512], F32, tag="xps")
                for jc in range(NJ):
                    pp = psA.tile([128, 512], F32, tag="ppA")
                    for kc in range(KP):
                        nc.tensor.matmul(
                            pp[:], wc[:, (kc * NJ + jc) * 128:(kc * NJ + jc + 1) * 128],
                            xt[kc][:], start=(kc == 0), stop=(kc == KP - 1))
                    stg = xtp.tile([128, 512], F32, tag=f"stgA{jc % 2}")
                    nc.vector.tensor_copy(stg[:], pp[:])
                    nc.sync.dma_start(scr[jc, :, :], stg[:])
                xps.append(scr)

            # ---- phase B: scan ----
            # h lives as fp16 slices of the output ring; tanh writes it once,
            # the next step's matmuls read it as the moving operand, and one
            # DMA per window ships it out. No copies.
            h_dt = _SCAN_MDT[SCAN_DT]
            HGRP = NJ // ELEM_CHAINS          # jc chunks per elementwise chain
            hz = hp.tile([128, NJ * BC], h_dt, tag="hzero")
            nc.vector.memset(hz[:], 0.0)
            hprev = hz
            for w in range(nb):
                xw = xwp.tile([128, NJ * 512], F32, tag="xw")
                nc.sync.dma_start(xw[:], xps[w][:].rearrange("j p n -> p j n"))
                oring = orp.tile([128, WIN * NJ * BC], h_dt, tag="oring")
                for trel in range(WIN):
                    P = psB.tile([128, NJ * BC], F32, tag="ppB")
                    for jc in range(NJ):
                        for kc in range(NJ):
                            nc.tensor.matmul(
                                P[:, jc * BC:(jc + 1) * BC],
                                wh[:, (kc * NJ + jc) * 128:(kc * NJ + jc + 1) * 128],
                                hprev[:, kc * BC:(kc + 1) * BC],
                                start=(kc == 0), stop=(kc == NJ - 1))
                    hnew = oring[:, trel * NJ * BC:(trel + 1) * NJ * BC]
                    for g in range(ELEM_CHAINS):
                        sl = slice(g * HGRP * BC, (g + 1) * HGRP * BC)
                        pg = P[:, sl].rearrange("p (c n) -> p c n", c=HGRP)
                        xg = xw[:].rearrange("p (c n) -> p c n", c=NJ)[
                            :, g * HGRP:(g + 1) * HGRP,
                            trel * BC:(trel + 1) * BC]
                        nc.vector.tensor_add(pg, pg, xg)
                        nc.scalar.activation(hnew[:, sl], P[:, sl], TANH)
                    hprev = hnew
                nc.sync.dma_start(
                    out_scr[:, w * WIN:(w + 1) * WIN, :], oring[:])
    nc.compile()
    return nc


def host_prep(x, constraints, W_ih, b_ih, W_hh, b_hh, W_ch, b_ch, t_steps=T_FULL):
    """Build per-core input maps."""
    x = np.asarray(x, dtype=np.float32)
    constraints = np.asarray(constraints, dtype=np.float32)
    bsz = x.shape[0]
    ncores_used = bsz // BC

    # augmented input  [b, t, 384]
    xcat = np.zeros((bsz, t_steps, KCAT), dtype=np.float32)
    xcat[:, :, :I] = x[:, :t_steps]
    xcat[:, :, I:I + C] = constraints[:, None, :]
    xcat[:, :, I + C] = 1.0

    # augmented weight [H, 384], tiles -> [128, KP*NJ*128]
    bias = (np.asarray(b_ih) + np.asarray(b_hh) + np.asarray(b_ch)).astype(np.float32)
    W_cat = np.zeros((H, KCAT), dtype=np.float32)
    W_cat[:, :I] = W_ih
    W_cat[:, I:I + C] = W_ch
    W_cat[:, I + C] = bias
    # wcat[p, kc*NJ+jc, m] = W_cat[jc*128+m, kc*128+p]
    wcat = np.ascontiguousarray(
        W_cat.T.reshape(KP, 128, NJ, 128).transpose(1, 0, 2, 3)
    ).reshape(128, KP * NJ * 128)

    # whh[p, kc*NJ+jc, m] = W_hh[jc*128+m, kc*128+p]
    whh = np.ascontiguousarray(
        np.asarray(W_hh, dtype=np.float32).T.reshape(NJ, 128, NJ, 128)
        .transpose(1, 0, 2, 3)
    ).reshape(128, 16 * 128)
    if SCAN_DT == "bf16":
        whh = whh.astype(ml_dtypes.bfloat16)
    elif SCAN_DT == "fp16":
        whh = whh.astype(np.float16)

    in_maps = []
    for c in range(ncores_used):
        xc = xcat[c * BC:(c + 1) * BC]                      # [16, t, 384]
        # xcatT[kc, ki, t*BC+bl] = xc[bl, t, kc*128+ki]
        xcT = np.ascontiguousarray(
            xc.reshape(BC, t_steps, KP, 128).transpose(2, 3, 1, 0)
        ).reshape(KP, 128, t_steps * BC)
        in_maps.append({"xcatT": xcT, "wcat": wcat, "whh": whh})
    return in_maps


def host_post(results, lengths, t_steps=T_FULL):
    """results: list of per-core dicts with out_scr [128, t, NJ*BC]."""
    lengths = np.asarray(lengths)
    outs = []
    for r in results:
        o = np.asarray(r["out_scr"], dtype=np.float32)      # [128, t, 4*16]
        o = o.reshape(128, t_steps, NJ, BC).transpose(3, 1, 2, 0)  # [bl,t,jc,p]
        outs.append(o.reshape(BC, t_steps, H))
    outputs = np.ascontiguousarray(np.concatenate(outs, axis=0), dtype=np.float32)
    bsz = outputs.shape[0]
    mask = (np.arange(t_steps)[None, :] < lengths[:bsz, None])
    outputs *= mask[:, :, None].astype(np.float32)
    last = outputs[np.arange(bsz), np.clip(lengths[:bsz], 1, t_steps) - 1]
    return outputs, np.ascontiguousarray(last, dtype=np.float32)


_NC_CACHE = {}


def _get_nc(t_steps):
    if t_steps not in _NC_CACHE:
        _NC_CACHE[t_steps] = build(t_steps)
    return _NC_CACHE[t_steps]


def run(x, constraints, lengths, W_ih, b_ih, W_hh, b_hh, W_ch, b_ch,
        t_steps=T_FULL, trace=False):
    nc = _get_nc(t_steps)
    in_maps = host_prep(x, constraints, W_ih, b_ih, W_hh, b_hh, W_ch, b_ch,
                        t_steps)
    res = run_bass_kernel_spmd(nc, in_maps, core_ids=list(range(len(in_maps))),
                               trace=trace)
    outputs, last = host_post(res.results, lengths, t_steps)
    return outputs, last, res


def kernel(x, constraints, lengths, W_ih, b_ih, W_hh, b_hh, W_ch, b_ch):
    outputs, last, _ = run(x, constraints, lengths, W_ih, b_ih, W_hh, b_hh,
                           W_ch, b_ch)
    return outputs, last


# revision 10
# speedup vs baseline: 2.0237x; 1.8855x over previous
"""Trainium2 Bass kernel for nn_ConstrainedRNN (B=128,T=1024,I=256,H=512,C=32).

Strategy (data-parallel over batch, 8 cores x 16 sequences):
  - Fold constraints/biases into one augmented input projection:
      pre[b,t,:] = [x[b,t] | constraints[b] | 1 | 0pad] @ W_cat.T
    computed as a large fp32r matmul (phase A) into DRAM scratch.
  - Sequential scan (phase B) in transposed layout hT[j, b]:
      hT_{t+1} = tanh(W_hhT-tiles (stationary, bf16) @ hT_t + xp_t)
    16 (ldweights+matmul) pairs per step with N=16 moving columns.
  - Masking / `last` extraction handled on host from `lengths` (positions
    t >= len never feed back into any used value, so the scan runs unmasked).
"""
import sys
import numpy as np

for _p in ("/opt/trn_rl_repo", "/root/.axon_site/_ro/trn_rl_repo"):
    if _p not in sys.path:
        sys.path.insert(0, _p)

import ml_dtypes
import concourse.bass as bass
import concourse.mybir as mybir
import concourse.tile as tile
from concourse import bacc
from concourse.bass_utils import run_bass_kernel_spmd

F32 = mybir.dt.float32
F32R = mybir.dt.float32r
BF16 = mybir.dt.bfloat16
FP16 = mybir.dt.float16
TANH = mybir.ActivationFunctionType.Tanh

B, T_FULL, I, H, C = 128, 1024, 256, 512, 32
NCORES = 8
BC = B // NCORES          # 16 sequences per core
NJ = H // 128             # 4 output-row chunks
KP = 3                    # projection contraction chunks (384 = 289 padded)
KCAT = KP * 128           # 384
WIN = 32                  # scan steps per window (= one 512-col projection block)

# dtype of the recurrent weight/moving operand:
#   "fp16" (fast, ~5e-4 weight rounding), "bf16" (fast, ~4e-3), "f32" (exact, slow)
SCAN_DT = "fp16"
_SCAN_MDT = {"fp16": FP16, "bf16": BF16, "f32": F32}
ELEM_CHAINS = 2           # elementwise chains per step (1, 2, or 4)


def build(t_steps=T_FULL):
    """Build the SPMD Bass module for one core (t_steps scan steps)."""
    cols = t_steps * BC
    nb = cols // 512          # number of 512-column blocks == number of windows
    assert t_steps % WIN == 0

    nc = bacc.Bacc("TRN2", target_bir_lowering=False)
    xcatT = nc.dram_tensor("xcatT", [KP, 128, cols], F32, kind="ExternalInput")
    wcat = nc.dram_tensor("wcat", [128, KP * NJ * 128], F32, kind="ExternalInput")
    whh_dt = _SCAN_MDT[SCAN_DT]
    whh = nc.dram_tensor("whh", [128, 16 * 128], whh_dt, kind="ExternalInput")
    ident = nc.dram_tensor("ident", [128, 128], whh_dt, kind="ExternalInput")
    out_scr = nc.dram_tensor("out_scr", [128, t_steps, NJ * BC],
                             _SCAN_MDT[SCAN_DT], kind="ExternalOutput")

    h_dt_s = _SCAN_MDT[SCAN_DT]
    with tile.TileContext(nc) as tc:
        with (
            tc.tile_pool(name="wpool", bufs=1) as wpool,
            tc.tile_pool(name="xtp", bufs=2 * KP) as xtp,
            tc.tile_pool(name="psA", bufs=2, space="PSUM") as psA,
            tc.tile_pool(name="dramp", bufs=nb, space="DRAM") as dramp,
            tc.tile_pool(name="xwp", bufs=2) as xwp,
            tc.tile_pool(name="orp", bufs=2) as orp,
            tc.tile_pool(name="psB", bufs=3, space="PSUM") as psB,
            tc.tile_pool(name="hp", bufs=1) as hp,
        ):
            # ---- persistent weights ----
            wc = wpool.tile([128, KP * NJ * 128], F32R, tag="wc")
            nc.gpsimd.dma_start(wc[:], wcat[:])          # f32 -> f32r cast DMA
            wh = wpool.tile([128, 16 * 128], whh_dt, tag="wh")
            nc.sync.dma_start(wh[:], whh[:])
            idt = wpool.tile([128, 128], whh_dt, tag="idt")
            nc.sync.dma_start(idt[:], ident[:])

            # ---- phase A: projection xp[jc, p, n] per 512-col block ----
            xps = []
            for ib in range(nb):
                xt = []
                for kc in range(KP):
                    xk = xtp.tile([128, 512], F32R, tag=f"xt{kc}")
                    nc.gpsimd.dma_start(
                        xk[:], xcatT[kc, :, ib * 512:(ib + 1) * 512])
                    xt.append(xk)
                scr = dramp.tile([NJ, 128, 512], h_dt_s, tag="xps")
                for jc in range(NJ):
                    pp = psA.tile([128, 512], F32, tag="ppA")
                    for kc in range(KP):
                        nc.tensor.matmul(
                            pp[:], wc[:, (kc * NJ + jc) * 128:(kc * NJ + jc + 1) * 128],
                            xt[kc][:], start=(kc == 0), stop=(kc == KP - 1))
                    stg = xtp.tile([128, 512], h_dt_s, tag=f"stgA{jc % 2}")
                    nc.vector.tensor_copy(stg[:], pp[:])
                    nc.sync.dma_start(scr[jc, :, :], stg[:])
                xps.append(scr)

            # ---- phase B: scan ----
            # h lives as fp16 slices of the output ring; tanh writes it once,
            # the next step's matmuls read it as the moving operand, and one
            # DMA per window ships it out. No copies.
            h_dt = _SCAN_MDT[SCAN_DT]
            HGRP = NJ // ELEM_CHAINS          # jc chunks per elementwise chain
            hz = hp.tile([128, NJ * BC], h_dt, tag="hzero")
            nc.vector.memset(hz[:], 0.0)
            hprev = hz
            for w in range(nb):
                xw = xwp.tile([128, NJ * 512], h_dt_s, tag="xw")
                nc.sync.dma_start(xw[:], xps[w][:].rearrange("j p n -> p j n"))
                oring = orp.tile([128, WIN * NJ * BC], h_dt, tag="oring")
                for trel in range(WIN):
                    hnew = oring[:, trel * NJ * BC:(trel + 1) * NJ * BC]
                    for g in range(ELEM_CHAINS):
                        # one psum tile (= one bank) per chain: a single
                        # start=True per bank (it clears the whole bank's
                        # has_written), everything after accumulates.
                        Pg = psB.tile([128, HGRP * BC], F32, tag=f"ppB{g}")
                        for ji, jc in enumerate(range(g * HGRP, (g + 1) * HGRP)):
                            for kc in range(NJ):
                                nc.tensor.matmul(
                                    Pg[:, ji * BC:(ji + 1) * BC],
                                    wh[:, (kc * NJ + jc) * 128:(kc * NJ + jc + 1) * 128],
                                    hprev[:, kc * BC:(kc + 1) * BC],
                                    start=(ji == 0 and kc == 0), stop=False,
                                    skip_group_check=True)
                        pg = Pg[:].rearrange("p (c n) -> p c n", c=HGRP)
                        xg = xw[:].rearrange("p (c n) -> p c n", c=NJ)[
                            :, g * HGRP:(g + 1) * HGRP,
                            trel * BC:(trel + 1) * BC]
                        nc.tensor.matmul(pg, idt[:], xg, start=False, stop=True,
                                         skip_group_check=True)
                        sl = slice(g * HGRP * BC, (g + 1) * HGRP * BC)
                        nc.scalar.activation(hnew[:, sl], Pg[:], TANH)
                    hprev = hnew
                nc.sync.dma_start(
                    out_scr[:, w * WIN:(w + 1) * WIN, :], oring[:])
    nc.compile()
    return nc


def host_prep(x, constraints, W_ih, b_ih, W_hh, b_hh, W_ch, b_ch, t_steps=T_FULL):
    """Build per-core input maps."""
    x = np.asarray(x, dtype=np.float32)
    constraints = np.asarray(constraints, dtype=np.float32)
    bsz = x.shape[0]
    ncores_used = bsz // BC

    # augmented input  [b, t, 384]
    xcat = np.zeros((bsz, t_steps, KCAT), dtype=np.float32)
    xcat[:, :, :I] = x[:, :t_steps]
    xcat[:, :, I:I + C] = constraints[:, None, :]
    xcat[:, :, I + C] = 1.0

    # augmented weight [H, 384], tiles -> [128, KP*NJ*128]
    bias = (np.asarray(b_ih) + np.asarray(b_hh) + np.asarray(b_ch)).astype(np.float32)
    W_cat = np.zeros((H, KCAT), dtype=np.float32)
    W_cat[:, :I] = W_ih
    W_cat[:, I:I + C] = W_ch
    W_cat[:, I + C] = bias
    # wcat[p, kc*NJ+jc, m] = W_cat[jc*128+m, kc*128+p]
    wcat = np.ascontiguousarray(
        W_cat.T.reshape(KP, 128, NJ, 128).transpose(1, 0, 2, 3)
    ).reshape(128, KP * NJ * 128)

    # whh[p, kc*NJ+jc, m] = W_hh[jc*128+m, kc*128+p]
    whh = np.ascontiguousarray(
        np.asarray(W_hh, dtype=np.float32).T.reshape(NJ, 128, NJ, 128)
        .transpose(1, 0, 2, 3)
    ).reshape(128, 16 * 128)
    if SCAN_DT == "bf16":
        whh = whh.astype(ml_dtypes.bfloat16)
    elif SCAN_DT == "fp16":
        whh = whh.astype(np.float16)

    npdt = {"fp16": np.float16, "bf16": ml_dtypes.bfloat16, "f32": np.float32}[SCAN_DT]
    ident = np.eye(128, dtype=np.float32).astype(npdt)
    in_maps = []
    for c in range(ncores_used):
        xc = xcat[c * BC:(c + 1) * BC]                      # [16, t, 384]
        # xcatT[kc, ki, t*BC+bl] = xc[bl, t, kc*128+ki]
        xcT = np.ascontiguousarray(
            xc.reshape(BC, t_steps, KP, 128).transpose(2, 3, 1, 0)
        ).reshape(KP, 128, t_steps * BC)
        in_maps.append({"xcatT": xcT, "wcat": wcat, "whh": whh, "ident": ident})
    return in_maps


def host_post(results, lengths, t_steps=T_FULL):
    """results: list of per-core dicts with out_scr [128, t, NJ*BC]."""
    lengths = np.asarray(lengths)
    outs = []
    for r in results:
        o = np.asarray(r["out_scr"], dtype=np.float32)      # [128, t, 4*16]
        o = o.reshape(128, t_steps, NJ, BC).transpose(3, 1, 2, 0)  # [bl,t,jc,p]
        outs.append(o.reshape(BC, t_steps, H))
    outputs = np.ascontiguousarray(np.concatenate(outs, axis=0), dtype=np.float32)
    bsz = outputs.shape[0]
    mask = (np.arange(t_steps)[None, :] < lengths[:bsz, None])
    outputs *= mask[:, :, None].astype(np.float32)
    last = outputs[np.arange(bsz), np.clip(lengths[:bsz], 1, t_steps) - 1]
    return outputs, np.ascontiguousarray(last, dtype=np.float32)


_NC_CACHE = {}


def _get_nc(t_steps):
    if t_steps not in _NC_CACHE:
        _NC_CACHE[t_steps] = build(t_steps)
    return _NC_CACHE[t_steps]


def run(x, constraints, lengths, W_ih, b_ih, W_hh, b_hh, W_ch, b_ch,
        t_steps=T_FULL, trace=False):
    nc = _get_nc(t_steps)
    in_maps = host_prep(x, constraints, W_ih, b_ih, W_hh, b_hh, W_ch, b_ch,
                        t_steps)
    res = run_bass_kernel_spmd(nc, in_maps, core_ids=list(range(len(in_maps))),
                               trace=trace)
    outputs, last = host_post(res.results, lengths, t_steps)
    return outputs, last, res


def kernel(x, constraints, lengths, W_ih, b_ih, W_hh, b_hh, W_ch, b_ch):
    outputs, last, _ = run(x, constraints, lengths, W_ih, b_ih, W_hh, b_hh,
                           W_ch, b_ch)
    return outputs, last


# revision 11
# speedup vs baseline: 2.0745x; 1.0251x over previous
"""Trainium2 Bass kernel for nn_ConstrainedRNN (B=128,T=1024,I=256,H=512,C=32).

Strategy (data-parallel over batch, 8 cores x 16 sequences):
  - Fold constraints/biases into one augmented input projection:
      pre[b,t,:] = [x[b,t] | constraints[b] | 1 | 0pad] @ W_cat.T
    computed as a large fp32r matmul (phase A) into DRAM scratch.
  - Sequential scan (phase B) in transposed layout hT[j, b]:
      hT_{t+1} = tanh(W_hhT-tiles (stationary, bf16) @ hT_t + xp_t)
    16 (ldweights+matmul) pairs per step with N=16 moving columns.
  - Masking / `last` extraction handled on host from `lengths` (positions
    t >= len never feed back into any used value, so the scan runs unmasked).
"""
import sys
import numpy as np

for _p in ("/opt/trn_rl_repo", "/root/.axon_site/_ro/trn_rl_repo"):
    if _p not in sys.path:
        sys.path.insert(0, _p)

import ml_dtypes
import concourse.bass as bass
import concourse.mybir as mybir
import concourse.tile as tile
from concourse import bacc
from concourse.bass_utils import run_bass_kernel_spmd

F32 = mybir.dt.float32
F32R = mybir.dt.float32r
BF16 = mybir.dt.bfloat16
FP16 = mybir.dt.float16
TANH = mybir.ActivationFunctionType.Tanh

B, T_FULL, I, H, C = 128, 1024, 256, 512, 32
NCORES = 8
BC = B // NCORES          # 16 sequences per core
NJ = H // 128             # 4 output-row chunks
KP = 3                    # projection contraction chunks (384 = 289 padded)
KCAT = KP * 128           # 384
WIN = 32                  # scan steps per window (= one 512-col projection block)

# dtype of the recurrent weight/moving operand:
#   "fp16" (fast, ~5e-4 weight rounding), "bf16" (fast, ~4e-3), "f32" (exact, slow)
SCAN_DT = "fp16"
_SCAN_MDT = {"fp16": FP16, "bf16": BF16, "f32": F32}
ELEM_CHAINS = 2           # elementwise chains per step (1, 2, or 4)


def build(t_steps=T_FULL):
    """Build the SPMD Bass module for one core (t_steps scan steps)."""
    cols = t_steps * BC
    nb = cols // 512          # number of 512-column blocks == number of windows
    assert t_steps % WIN == 0

    nc = bacc.Bacc("TRN2", target_bir_lowering=False)
    xcatT = nc.dram_tensor("xcatT", [KP, 128, cols], F32, kind="ExternalInput")
    wcat = nc.dram_tensor("wcat", [128, KP * NJ * 128], F32, kind="ExternalInput")
    whh_dt = _SCAN_MDT[SCAN_DT]
    whh = nc.dram_tensor("whh", [128, 16 * 128], whh_dt, kind="ExternalInput")
    ident = nc.dram_tensor("ident", [128, 128], whh_dt, kind="ExternalInput")
    out_scr = nc.dram_tensor("out_scr", [128, t_steps, NJ * BC],
                             _SCAN_MDT[SCAN_DT], kind="ExternalOutput")

    h_dt_s = _SCAN_MDT[SCAN_DT]
    with tile.TileContext(nc) as tc:
        with (
            tc.tile_pool(name="wpool", bufs=1) as wpool,
            tc.tile_pool(name="xtp", bufs=2) as xtp,
            tc.tile_pool(name="psA", bufs=2, space="PSUM") as psA,
            tc.tile_pool(name="xpres", bufs=nb) as xpres,
            tc.tile_pool(name="orp", bufs=2) as orp,
            tc.tile_pool(name="psB", bufs=3, space="PSUM") as psB,
            tc.tile_pool(name="hp", bufs=1) as hp,
        ):
            # ---- persistent weights ----
            wc = wpool.tile([128, KP * NJ * 128], F32R, tag="wc")
            nc.gpsimd.dma_start(wc[:], wcat[:])          # f32 -> f32r cast DMA
            wh = wpool.tile([128, 16 * 128], whh_dt, tag="wh")
            nc.sync.dma_start(wh[:], whh[:])
            idt = wpool.tile([128, 128], whh_dt, tag="idt")
            nc.sync.dma_start(idt[:], ident[:])

            # projection of one 512-col block into an SBUF-resident fp16 tile
            # [128, NJ*512] with layout f = jc*512 + n.
            xps = []

            def project_block(ib):
                xt = []
                for kc in range(KP):
                    xk = xtp.tile([128, 512], F32R, tag=f"xt{kc}")
                    nc.gpsimd.dma_start(
                        xk[:], xcatT[kc, :, ib * 512:(ib + 1) * 512])
                    xt.append(xk)
                res = xpres.tile([128, NJ * 512], h_dt_s, tag="xpres")
                for jc in range(NJ):
                    pp = psA.tile([128, 512], F32, tag="ppA")
                    for kc in range(KP):
                        nc.tensor.matmul(
                            pp[:], wc[:, (kc * NJ + jc) * 128:(kc * NJ + jc + 1) * 128],
                            xt[kc][:], start=(kc == 0), stop=(kc == KP - 1))
                    nc.vector.tensor_copy(
                        res[:, jc * 512:(jc + 1) * 512], pp[:])
                xps.append(res)

            # ---- scan, with block projections interleaved two windows ahead
            h_dt = _SCAN_MDT[SCAN_DT]
            HGRP = NJ // ELEM_CHAINS          # jc chunks per elementwise chain
            hz = hp.tile([128, NJ * BC], h_dt, tag="hzero")
            nc.vector.memset(hz[:], 0.0)
            hprev = hz
            project_block(0)
            if nb > 1:
                project_block(1)
            for w in range(nb):
                if w + 2 < nb:
                    project_block(w + 2)
                xw = xps[w]
                oring = orp.tile([128, WIN * NJ * BC], h_dt, tag="oring")
                for trel in range(WIN):
                    hnew = oring[:, trel * NJ * BC:(trel + 1) * NJ * BC]
                    for g in range(ELEM_CHAINS):
                        # one psum tile (= one bank) per chain: a single
                        # start=True per bank (it clears the whole bank's
                        # has_written), everything after accumulates.
                        Pg = psB.tile([128, HGRP * BC], F32, tag=f"ppB{g}")
                        for ji, jc in enumerate(range(g * HGRP, (g + 1) * HGRP)):
                            for kc in range(NJ):
                                nc.tensor.matmul(
                                    Pg[:, ji * BC:(ji + 1) * BC],
                                    wh[:, (kc * NJ + jc) * 128:(kc * NJ + jc + 1) * 128],
                                    hprev[:, kc * BC:(kc + 1) * BC],
                                    start=(ji == 0 and kc == 0), stop=False,
                                    skip_group_check=True)
                        pg = Pg[:].rearrange("p (c n) -> p c n", c=HGRP)
                        xg = xw[:].rearrange("p (c n) -> p c n", c=NJ)[
                            :, g * HGRP:(g + 1) * HGRP,
                            trel * BC:(trel + 1) * BC]
                        nc.tensor.matmul(pg, idt[:], xg, start=False, stop=True,
                                         skip_group_check=True)
                        sl = slice(g * HGRP * BC, (g + 1) * HGRP * BC)
                        nc.scalar.activation(hnew[:, sl], Pg[:], TANH)
                    hprev = hnew
                nc.sync.dma_start(
                    out_scr[:, w * WIN:(w + 1) * WIN, :], oring[:])
    nc.compile()
    return nc


def host_prep(x, constraints, W_ih, b_ih, W_hh, b_hh, W_ch, b_ch, t_steps=T_FULL):
    """Build per-core input maps."""
    x = np.asarray(x, dtype=np.float32)
    constraints = np.asarray(constraints, dtype=np.float32)
    bsz = x.shape[0]
    ncores_used = bsz // BC

    # augmented input  [b, t, 384]
    xcat = np.zeros((bsz, t_steps, KCAT), dtype=np.float32)
    xcat[:, :, :I] = x[:, :t_steps]
    xcat[:, :, I:I + C] = constraints[:, None, :]
    xcat[:, :, I + C] = 1.0

    # augmented weight [H, 384], tiles -> [128, KP*NJ*128]
    bias = (np.asarray(b_ih) + np.asarray(b_hh) + np.asarray(b_ch)).astype(np.float32)
    W_cat = np.zeros((H, KCAT), dtype=np.float32)
    W_cat[:, :I] = W_ih
    W_cat[:, I:I + C] = W_ch
    W_cat[:, I + C] = bias
    # wcat[p, kc*NJ+jc, m] = W_cat[jc*128+m, kc*128+p]
    wcat = np.ascontiguousarray(
        W_cat.T.reshape(KP, 128, NJ, 128).transpose(1, 0, 2, 3)
    ).reshape(128, KP * NJ * 128)

    # whh[p, kc*NJ+jc, m] = W_hh[jc*128+m, kc*128+p]
    whh = np.ascontiguousarray(
        np.asarray(W_hh, dtype=np.float32).T.reshape(NJ, 128, NJ, 128)
        .transpose(1, 0, 2, 3)
    ).reshape(128, 16 * 128)
    if SCAN_DT == "bf16":
        whh = whh.astype(ml_dtypes.bfloat16)
    elif SCAN_DT == "fp16":
        whh = whh.astype(np.float16)

    npdt = {"fp16": np.float16, "bf16": ml_dtypes.bfloat16, "f32": np.float32}[SCAN_DT]
    ident = np.eye(128, dtype=np.float32).astype(npdt)
    in_maps = []
    for c in range(ncores_used):
        xc = xcat[c * BC:(c + 1) * BC]                      # [16, t, 384]
        # xcatT[kc, ki, t*BC+bl] = xc[bl, t, kc*128+ki]
        xcT = np.ascontiguousarray(
            xc.reshape(BC, t_steps, KP, 128).transpose(2, 3, 1, 0)
        ).reshape(KP, 128, t_steps * BC)
        in_maps.append({"xcatT": xcT, "wcat": wcat, "whh": whh, "ident": ident})
    return in_maps


def host_post(results, lengths, t_steps=T_FULL):
    """results: list of per-core dicts with out_scr [128, t, NJ*BC]."""
    lengths = np.asarray(lengths)
    outs = []
    for r in results:
        o = np.asarray(r["out_scr"], dtype=np.float32)      # [128, t, 4*16]
        o = o.reshape(128, t_steps, NJ, BC).transpose(3, 1, 2, 0)  # [bl,t,jc,p]
        outs.append(o.reshape(BC, t_steps, H))
    outputs = np.ascontiguousarray(np.concatenate(outs, axis=0), dtype=np.float32)
    bsz = outputs.shape[0]
    mask = (np.arange(t_steps)[None, :] < lengths[:bsz, None])
    outputs *= mask[:, :, None].astype(np.float32)
    last = outputs[np.arange(bsz), np.clip(lengths[:bsz], 1, t_steps) - 1]
    return outputs, np.ascontiguousarray(last, dtype=np.float32)


_NC_CACHE = {}


def _get_nc(t_steps):
    if t_steps not in _NC_CACHE:
        _NC_CACHE[t_steps] = build(t_steps)
    return _NC_CACHE[t_steps]


def run(x, constraints, lengths, W_ih, b_ih, W_hh, b_hh, W_ch, b_ch,
        t_steps=T_FULL, trace=False):
    nc = _get_nc(t_steps)
    in_maps = host_prep(x, constraints, W_ih, b_ih, W_hh, b_hh, W_ch, b_ch,
                        t_steps)
    res = run_bass_kernel_spmd(nc, in_maps, core_ids=list(range(len(in_maps))),
                               trace=trace)
    outputs, last = host_post(res.results, lengths, t_steps)
    return outputs, last, res


def kernel(x, constraints, lengths, W_ih, b_ih, W_hh, b_hh, W_ch, b_ch):
    outputs, last, _ = run(x, constraints, lengths, W_ih, b_ih, W_hh, b_hh,
                           W_ch, b_ch)
    return outputs, last
